# revision 13
# baseline (speedup 1.0000x reference)
"""Multi-head attention (B=2, S=2048, D=1024, H=16) on 8 TRN2 NeuronCores.

Tensor-parallel over heads: core c owns heads {2c, 2c+1} (a 128-wide slice of
the QKV projections / a 128-row slice of Wo). Each core computes its partial
out-projection in fp16; the host sums the 8 partials and adds the bias.

v2 layout (all-bf16/fp16 matmuls, q-major AV):
  - QT/KT = (q @ Wq|k)^T in [dh2h, bs] layout (lhsT = W chunk, rhs = qT chunk)
  - V projected directly into vh [s(=k), dv2h] tiles (lhsT = qT chunk)
  - scores k-major: sT[k, q] = KT_h-slice.T @ QT_h-slice, psum [128, 1024]
  - exp on ScalarE (fused 1/8 scale, no max subtraction; scores ~N(0,1)),
    strip bf16 in SBUF
  - AV q-major: ctx[q, dv] += strip_slice.T @ vh_slice accumulated over the
    16 k-tiles; row-sums via an extra ones-column matmul per q-tile (out
    free size 1 -> ~free)
  - normalization fused into the ctx transpose: a regular matmul against
    diag(1/D) (built by GPSIMD from an fp16 identity) yields
    ctxT[dvh, q] = ctx[q, dvh]/D_q
  - out projection: po[q, e] = ctxT.T-slice @ Wo-slice, fp16 partial out

Scheduling: a global 128-slot pipeline (slot = one (pass, k-tile)); each slot
carries scores+exp+AV(lag 3) plus "filler" PE work (projections, V, out-proj)
budgeted so the tensor engine never idles (the cost model's p-state ramp
penalizes every PE idle gap). Warmup dummy matmuls cover the initial DMA wait.
"""

import numpy as np
import ml_dtypes

import concourse.bass as bass
import concourse.mybir as mybir
import concourse.tile as tile
from concourse import bacc
from concourse.bass_utils import run_bass_kernel_spmd
from concourse.masks import make_identity

BF16 = mybir.dt.bfloat16
FP16 = mybir.dt.float16
F32 = mybir.dt.float32
EXP = mybir.ActivationFunctionType.Exp

B, S, D = 2, 2048, 1024
H, DH = 16, 64
NCORES = 8
BS = B * S  # 4096
NCH = D // 128  # 8 contraction chunks for the projections
NKT = S // 128  # 16 k-tiles per batch
NQT = 8  # q-tiles (128) per q-half
AV_LAG = 3  # slots between exp(s) and AV(s)
N_DUMMY = 10  # warmup matmuls riding out the DMA wait + p-state ramp

# pass p = (b, qh, h); slot s = p*16 + kt
PASSES = [(b, qh, h) for b in range(B) for qh in range(2) for h in range(2)]
NSLOT = len(PASSES) * NKT  # 128

_CACHED_NC = None


def _build():
    nc = bacc.Bacc("TRN2", target_bir_lowering=False, debug=False)

    qt_d = nc.dram_tensor("qt", [128, NCH, BS], BF16, kind="ExternalInput")
    wq_d = nc.dram_tensor("wq", [128, NCH, 128], BF16, kind="ExternalInput")
    wk_d = nc.dram_tensor("wk", [128, NCH, 128], BF16, kind="ExternalInput")
    wv_d = nc.dram_tensor("wv", [128, NCH, 128], BF16, kind="ExternalInput")
    wo_d = nc.dram_tensor("wo", [128, D], FP16, kind="ExternalInput")
    out_d = nc.dram_tensor("out", [BS, D], FP16, kind="ExternalOutput")

    with tile.TileContext(nc) as tc:
        with (
            tc.tile_pool(name="cp", bufs=1) as cp,
            tc.tile_pool(name="sp", bufs=1) as sp,
            tc.tile_pool(name="ps", bufs=1, space="PSUM") as ps,
        ):
            # ---- persistent SBUF ----
            qt_sb = cp.tile([128, NCH, BS], BF16, tag="qt")
            wq_sb = cp.tile([128, NCH, 128], BF16, tag="wq")
            wk_sb = cp.tile([128, NCH, 128], BF16, tag="wk")
            wv_sb = cp.tile([128, NCH, 128], BF16, tag="wv")
            wo_sb = cp.tile([128, D], FP16, tag="wo")
            QTt = cp.tile([128, BS], BF16, tag="QT")  # [2h*dh, b*s]
            KTt = cp.tile([128, BS], BF16, tag="KT")
            vht = cp.tile([128, B, NKT, 128], BF16, tag="vh")  # [k, b, kt, dv2h]
            onesc = cp.tile([128, 1], BF16, tag="ones")
            identf = cp.tile([128, 128], FP16, tag="ident")
            dmy_w = cp.tile([128, 128], BF16, tag="dmy_w")
            dmy_a = cp.tile([128, 384], BF16, tag="dmy_a")

            # one shared PSUM bank for the small tiles (bank-granular
            # allocator): per-kt D partials [*,0:128] as [8qt,16kt], vv
            # [*,128:256], ptr ping-pong [*,256:384] / [*,384:512]
            smallp = ps.tile([128, 512], F32, tag="small", bufs=1)

            nc.vector.memset(onesc[:], 1.0)
            nc.vector.memset(dmy_w[:], 0.0)
            nc.vector.memset(dmy_a[:], 0.0)
            make_identity(nc, identf[:])

            # ---- warmup dummies (PE busy during DMA wait; ride the ramp) ----
            for i in range(N_DUMMY):
                pd = ps.tile(
                    [128, 1024], F32, tag="pscr", bufs=2, name=f"dmy{i}"
                )
                nc.tensor.matmul(
                    pd[:, 0:384], dmy_w[:], dmy_a[:], start=True, stop=True
                )

            # ---- input DMAs: critical wave first, issue spread across
            # the SP/ACT/DVE DGE queues (issue itself serializes per engine)
            nc.sync.dma_start(wq_sb[:], wq_d.ap())
            nc.scalar.dma_start(wk_sb[:], wk_d.ap())
            nc.sync.dma_start(qt_sb[:, 0:4, 0:512], qt_d[:, 0:4, 0:512])
            nc.scalar.dma_start(qt_sb[:, 4:8, 0:512], qt_d[:, 4:8, 0:512])
            nc.sync.dma_start(qt_sb[:, 0:4, 512:1024], qt_d[:, 0:4, 512:1024])
            nc.scalar.dma_start(qt_sb[:, 4:8, 512:1024], qt_d[:, 4:8, 512:1024])
            nc.sync.dma_start(wv_sb[:], wv_d.ap())
            nc.scalar.dma_start(qt_sb[:, 4:8, 1024:2048], qt_d[:, 4:8, 1024:2048])
            nc.sync.dma_start(qt_sb[:, 0:4, 1024:2048], qt_d[:, 0:4, 1024:2048])
            nc.scalar.dma_start(qt_sb[:, 4:8, 2048:3072], qt_d[:, 4:8, 2048:3072])
            nc.sync.dma_start(qt_sb[:, 0:4, 2048:3072], qt_d[:, 0:4, 2048:3072])
            nc.scalar.dma_start(qt_sb[:, 4:8, 3072:BS], qt_d[:, 4:8, 3072:BS])
            nc.sync.dma_start(qt_sb[:, 0:4, 3072:BS], qt_d[:, 0:4, 3072:BS])
            nc.scalar.dma_start(wo_sb[:], wo_d[:, :])

            # ---- helpers ----
            def proj_half(which, blk, half):
                """4 of the 8 contraction-chunk matmuls of one 512-col
                projection block. Each half owns its flex psum tile for its
                whole lifecycle (alloc->mms->evict) so the shared flex ring
                can never deadlock the PE FIFO. half 1 adds into the dst."""
                w_sb, dst = (wq_sb, QTt) if which == "Q" else (wk_sb, KTt)
                s0 = blk * 512
                pt = ps.tile(
                    [128, 512], F32, tag="flex", bufs=2, name=f"pt_{which}{blk}_{half}"
                )
                for c in range(half * 4, half * 4 + 4):
                    nc.tensor.matmul(
                        pt[:],
                        w_sb[:, c, :],
                        qt_sb[:, c, s0 : s0 + 512],
                        start=(c == half * 4),
                        stop=(c == half * 4 + 3),
                    )
                if half == 0:
                    nc.vector.tensor_copy(dst[:, s0 : s0 + 512], pt[:])
                else:
                    nc.vector.tensor_tensor(
                        dst[:, s0 : s0 + 512],
                        dst[:, s0 : s0 + 512],
                        pt[:],
                        mybir.AluOpType.add,
                    )

            def v_tile(bb, st):
                """Project one 128-row s-tile of batch bb into vh."""
                vv = smallp[:, 128:256]
                s0 = bb * S + st * 128
                for c in range(NCH):
                    nc.tensor.matmul(
                        vv[:],
                        qt_sb[:, c, s0 : s0 + 128],
                        wv_sb[:, c, :],
                        start=(c == 0),
                        stop=(c == 7),
                    )
                nc.vector.tensor_copy(vht[:, bb, st, :], vv[:])

            cc_tiles = {}
            rcp_tiles = {}
            diag_tiles = {}
            ptr_idx = [0]

            def scores_exp(b, qh, h, kt, s):
                hp = h * 64
                k0 = b * S + kt * 128
                q0 = b * S + qh * 1024
                pscr = ps.tile([128, 1024], F32, tag="pscr", bufs=2, name=f"sc{s}")
                for j in range(2):
                    nc.tensor.matmul(
                        pscr[:, j * 512 : (j + 1) * 512],
                        KTt[hp : hp + 64, k0 : k0 + 128],
                        QTt[hp : hp + 64, q0 + j * 512 : q0 + (j + 1) * 512],
                        start=True,
                        stop=True,
                    )
                strip = sp.tile(
                    [128, 1024], BF16, tag="strip", bufs=32, name=f"st{s}"
                )
                nc.scalar.activation(strip[:], pscr[:], EXP, scale=0.125)
                return strip

            av_state = {}
            strips = {}

            def av_qt(p, qtt):
                """Full 16-kt accumulation for one q-tile of pass p (runs as
                a filler during pass p+1). Groups are contiguous: interleaved
                PSUM accumulation groups in one bank break (bank-level
                has_written clear on start)."""
                b, qh, h = PASSES[p]
                if qtt == 0:
                    av_state[p] = ps.tile(
                        [128, NQT, 64], F32, tag="avp", bufs=1, name=f"av{p}"
                    )
                avp = av_state[p]
                for kt in range(NKT):
                    nc.tensor.matmul(
                        avp[:, qtt, :],
                        strips[p * 16 + kt][:, qtt * 128 : (qtt + 1) * 128],
                        vht[:, b, kt, h * 64 : h * 64 + 64],
                        start=(kt == 0),
                        stop=(kt == NKT - 1),
                    )
                for kt in range(NKT):
                    nc.tensor.matmul(
                        smallp[:, qtt : qtt + 1],
                        strips[p * 16 + kt][:, qtt * 128 : (qtt + 1) * 128],
                        onesc[:],
                        start=(kt == 0),
                        stop=(kt == NKT - 1),
                    )
                if qtt == NQT - 1:
                    for kt in range(NKT):
                        strips.pop(p * 16 + kt)
                    drain(p)

            def drain(p):
                """recip of the row-sums + evacuate ctx into cc (SBUF fp16)."""
                b, qh, h = PASSES[p]
                avp = av_state.pop(p)
                if h == 0:
                    cc_tiles[(b, qh)] = cp.tile(
                        [128, NQT, 128], FP16, tag="cc", bufs=4, name=f"cc{b}{qh}"
                    )
                    rcp_tiles[(b, qh)] = cp.tile(
                        [128, 2, NQT], F32, tag="rcp", bufs=4, name=f"rcp{b}{qh}"
                    )
                cc = cc_tiles[(b, qh)]
                rcp = rcp_tiles[(b, qh)]
                with nc.allow_low_precision(reason="softmax denominator"):
                    nc.vector.reciprocal(rcp[:, h, :], smallp[:, 0:8])
                nc.vector.tensor_copy(cc[:, :, h * 64 : h * 64 + 64], avp[:])
                if h == 1:
                    # pre-build all diag(1/D) tiles on GPSIMD so the
                    # out-projection transposes never wait on Pool
                    dgs = []
                    for qtt in range(NQT):
                        for hh in range(2):
                            dg = sp.tile(
                                [128, 128], FP16, tag="diag", bufs=36,
                                name=f"dg{b}{qh}{qtt}{hh}",
                            )
                            nc.gpsimd.tensor_scalar_mul(
                                dg[:], identf[:], rcp[:, hh, qtt : qtt + 1]
                            )
                            dgs.append(dg)
                    diag_tiles[(b, qh)] = dgs

            def outproj_a(b, qh, qtt):
                """transpose-normalize matmuls (diag pre-built) + ctxT copy."""
                cc = cc_tiles[(b, qh)]
                pp = ptr_idx[0] % 2
                ptr_idx[0] += 1
                ptr = smallp[:, 256 + pp * 128 : 384 + pp * 128]
                for h in range(2):
                    dg = diag_tiles[(b, qh)][qtt * 2 + h]
                    nc.tensor.matmul(
                        ptr[h * 64 : (h + 1) * 64, :],
                        cc[:, qtt, h * 64 : (h + 1) * 64],
                        dg[:],
                        start=True,
                        stop=True,
                    )
                ctxT = sp.tile(
                    [128, 128], FP16, tag="ctxT", bufs=4, name=f"cx{b}{qh}{qtt}"
                )
                nc.vector.tensor_copy(ctxT[:], ptr[:])
                return ctxT

            def outproj_b(b, qh, qtt, ctxT, eh, tail=False):
                po = ps.tile(
                    [128, 512], F32, tag="flex", bufs=2, name=f"po{b}{qh}{qtt}{eh}"
                )
                nc.tensor.matmul(
                    po[:],
                    ctxT[:],
                    wo_sb[:, eh * 512 : (eh + 1) * 512],
                    start=True,
                    stop=True,
                )
                ob = sp.tile(
                    [128, 512], FP16, tag="ob", bufs=3, name=f"ob{b}{qh}{qtt}{eh}"
                )
                if tail and eh == 1:
                    nc.scalar.copy(ob[:], po[:])  # ACT is free in the tail
                else:
                    nc.vector.tensor_copy(ob[:], po[:])
                r0 = b * S + qh * 1024 + qtt * 128
                nc.sync.dma_start(
                    out_d[r0 : r0 + 128, eh * 512 : (eh + 1) * 512], ob[:]
                )

            # ---- filler schedule ----
            # Each filler is (ready_slot, deadline_slot, fn). Deadlines are
            # asserted; ready respects data deps (proj blocks are sequential
            # through the single pt psum tile by construction of the list).
            fillers = []

            def F(ready, deadline, fn, cost=450):
                fillers.append([ready, deadline, fn, cost])

            # KT b0 remaining blocks (blk0 in prologue): scores kt needs KT
            # block kt//4 at slot kt.
            for blk, dl in ((1, 4), (2, 8), (3, 12)):
                for hh in range(2):
                    F(0, dl - 1, lambda blk=blk, hh=hh: proj_half("K", blk, hh), 853)
            # V b0: vh[st] needed by AV kt=st at slot st+AV_LAG (st0,1 in prologue)
            for st in range(2, 16):
                F(0, 14, lambda st=st: v_tile(0, st), 427)
            # QT b0 blk2,3 (qh1 -> slot 32)
            for blk in (2, 3):
                for hh in range(2):
                    F(4, 31, lambda blk=blk, hh=hh: proj_half("Q", blk, hh), 853)
            # b1 projections (scores from slot 64; KT block (kt//4)+4 at slot
            # 64+kt; QT blk4,5 by 64, blk6,7 by 96)
            for blk, dl in ((4, 62), (5, 66), (6, 70), (7, 74)):
                for hh in range(2):
                    F(16, dl, lambda blk=blk, hh=hh: proj_half("K", blk, hh), 853)
            for blk, dl in ((4, 62), (5, 62), (6, 93), (7, 93)):
                for hh in range(2):
                    F(
                        18,
                        dl,
                        lambda blk=blk, hh=hh: proj_half("Q", blk, hh),
                        853,
                    )
            # V b1: needed from slot 64+st+AV_LAG
            for st in range(16):
                F(40, 78, lambda st=st: v_tile(1, st), 427)

            # out-projections become ready once both h-passes of (b, qh) have
            # drained: pass p=(b,qh,1) drains at slot p*16+15+AV_LAG.
            ctxT_holder = {}

            def op_a(b, qh, qtt):
                ctxT_holder[(b, qh, qtt)] = outproj_a(b, qh, qtt)

            def op_b(b, qh, qtt, eh):
                outproj_b(b, qh, qtt, ctxT_holder[(b, qh, qtt)], eh)

            # deferred AV: pass p's per-q-tile accumulations run during
            # pass p+1 (they need all 16 strips of pass p)
            for p in range(len(PASSES) - 1):
                for qtt in range(NQT):
                    F(
                        (p + 1) * 16 + qtt,
                        (p + 1) * 16 + 13,
                        lambda p=p, qtt=qtt: av_qt(p, qtt),
                        500,
                    )

            for gi, (b, qh) in enumerate([(0, 0), (0, 1), (1, 0)]):
                rdy = (PASSES.index((b, qh, 1)) + 1) * 16 + 15
                for qtt in range(NQT):
                    F(rdy + 2 * qtt, NSLOT - 1, lambda b=b, qh=qh, qtt=qtt: op_a(b, qh, qtt), 260)
                for qtt in range(NQT):
                    F(rdy + 2 * qtt + 2, NSLOT - 1, lambda b=b, qh=qh, qtt=qtt: op_b(b, qh, qtt, 0), 260)
                    F(rdy + 2 * qtt + 3, NSLOT - 1, lambda b=b, qh=qh, qtt=qtt: op_b(b, qh, qtt, 1), 260)

            fillers.sort(key=lambda f: (f[0], f[1]))

            # ---- prologue projections (needed before slot 0) ----
            for hh in range(2):
                proj_half("Q", 0, hh)
            for hh in range(2):
                proj_half("K", 0, hh)
            for hh in range(2):
                proj_half("Q", 1, hh)
            v_tile(0, 0)
            v_tile(0, 1)

            # ---- main pipeline ----
            done = [False] * len(fillers)

            def pop_fillers(s, budget):
                """Emit due/ready fillers for slot s up to ~budget ns of PE
                work (earliest-deadline-first among ready; deadline-urgent
                entries are always emitted)."""
                spent = 0
                ready = [
                    (fillers[j][1], j)
                    for j in range(len(fillers))
                    if not done[j] and fillers[j][0] <= s
                ]
                ready.sort()
                for _, idx in ready:
                    rdy, dl, fn, cost = fillers[idx]
                    urgent = dl <= s + 2
                    if not urgent and spent + cost > budget + 200:
                        break
                    fn()
                    done[idx] = True
                    spent += cost
                return spent

            for s in range(NSLOT):
                p, kt = divmod(s, NKT)
                b, qh, h = PASSES[p]
                strips[s] = scores_exp(b, qh, h, kt, s)
                pop_fillers(s, budget=650)

            # ---- epilogue: phase 1 (AV/D + recip + cc + diags), then
            # phase 2 (transpose-normalize + out-proj), so PE never waits on
            # the DVE/Pool round-trips
            for idx, (rdy, dl, fn, cost) in enumerate(fillers):
                if not done[idx]:
                    fn()
                    done[idx] = True
            p7 = len(PASSES) - 1
            b7, qh7, h7 = PASSES[p7]
            cc7 = cc_tiles[(b7, qh7)]
            rcp7 = rcp_tiles[(b7, qh7)]
            avp7 = ps.tile([128, NQT, 64], F32, tag="avp", bufs=1, name="av7")
            dgs7 = []
            for qtt in range(NQT):
                for kt in range(NKT):
                    nc.tensor.matmul(
                        avp7[:, qtt, :],
                        strips[p7 * 16 + kt][:, qtt * 128 : (qtt + 1) * 128],
                        vht[:, b7, kt, h7 * 64 : h7 * 64 + 64],
                        start=(kt == 0),
                        stop=(kt == NKT - 1),
                    )
                for kt in range(NKT):
                    nc.tensor.matmul(
                        smallp[:, qtt : qtt + 1],
                        strips[p7 * 16 + kt][:, qtt * 128 : (qtt + 1) * 128],
                        onesc[:],
                        start=(kt == 0),
                        stop=(kt == NKT - 1),
                    )
                with nc.allow_low_precision(reason="softmax denominator"):
                    nc.vector.reciprocal(
                        rcp7[:, h7, qtt : qtt + 1], smallp[:, qtt : qtt + 1]
                    )
                nc.vector.tensor_copy(
                    cc7[:, qtt, h7 * 64 : h7 * 64 + 64], avp7[:, qtt, :]
                )
                for hh in range(2):
                    dg = sp.tile(
                        [128, 128], FP16, tag="diag", bufs=36, name=f"edg{qtt}{hh}"
                    )
                    nc.gpsimd.tensor_scalar_mul(
                        dg[:], identf[:], rcp7[:, hh, qtt : qtt + 1]
                    )
                    dgs7.append(dg)
            for qtt in range(NQT):
                pp = ptr_idx[0] % 2
                ptr_idx[0] += 1
                ptr = smallp[:, 256 + pp * 128 : 384 + pp * 128]
                for h in range(2):
                    nc.tensor.matmul(
                        ptr[h * 64 : (h + 1) * 64, :],
                        cc7[:, qtt, h * 64 : (h + 1) * 64],
                        dgs7[qtt * 2 + h][:],
                        start=True,
                        stop=True,
                    )
                ctxT = sp.tile([128, 128], FP16, tag="ctxT", bufs=4, name=f"ecx{qtt}")
                nc.vector.tensor_copy(ctxT[:], ptr[:])
                po = ps.tile([128, 1024], F32, tag="pscr", bufs=2, name=f"epo{qtt}")
                for eh in range(2):
                    nc.tensor.matmul(
                        po[:, eh * 512 : (eh + 1) * 512],
                        ctxT[:],
                        wo_sb[:, eh * 512 : (eh + 1) * 512],
                        start=True,
                        stop=True,
                    )
                ob = sp.tile([128, 1024], FP16, tag="eob", bufs=3, name=f"eob{qtt}")
                nc.vector.tensor_copy(ob[:, 0:512], po[:, 0:512])
                nc.scalar.copy(ob[:, 512:1024], po[:, 512:1024])
                r0 = b7 * S + qh7 * 1024 + qtt * 128
                nc.sync.dma_start(out_d[r0 : r0 + 128, :], ob[:])
            for kt in range(NKT):
                strips.pop(p7 * 16 + kt)

    nc.compile()
    return nc


def _get_nc():
    global _CACHED_NC
    if _CACHED_NC is None:
        _CACHED_NC = _build()
    return _CACHED_NC


def _in_maps(q, Wq, Wk, Wv, Wo):
    q32 = np.asarray(q, np.float32).reshape(BS, NCH, 128)
    qt = np.ascontiguousarray(q32.transpose(2, 1, 0)).astype(ml_dtypes.bfloat16)

    def warr(W, sl):
        w = np.asarray(W, np.float32)[:, sl].reshape(NCH, 128, 128).transpose(1, 0, 2)
        return np.ascontiguousarray(w).astype(ml_dtypes.bfloat16)

    Wo32 = np.asarray(Wo, np.float32)
    maps = []
    for c in range(NCORES):
        sl = slice(c * 128, (c + 1) * 128)
        maps.append(
            {
                "qt": qt,
                "wq": warr(Wq, sl),
                "wk": warr(Wk, sl),
                "wv": warr(Wv, sl),
                "wo": np.ascontiguousarray(Wo32[sl, :]).astype(np.float16),
            }
        )
    return maps


def run(q, Wq, Wk, Wv, Wo, bo, trace=False):
    nc = _get_nc()
    res = run_bass_kernel_spmd(
        nc, _in_maps(q, Wq, Wk, Wv, Wo), list(range(NCORES)), trace=trace
    )
    acc = np.zeros((BS, D), np.float32)
    for r in res.results:
        acc += r["out"].astype(np.float32)
    out = (acc + np.asarray(bo, np.float32)).astype(np.float32)
    return out.reshape(B, S, D), res


def kernel(q, Wq, Wk, Wv, Wo, bo):
    out, _ = run(q, Wq, Wk, Wv, Wo, bo)
    return out


# revision 14
# speedup vs baseline: 1.0052x; 1.0052x over previous
"""Multi-head attention (B=2, S=2048, D=1024, H=16) on 8 TRN2 NeuronCores.

Tensor-parallel over heads: core c owns heads {2c, 2c+1} (a 128-wide slice of
the QKV projections / a 128-row slice of Wo). Each core computes its partial
out-projection in fp16; the host sums the 8 partials and adds the bias.

v2 layout (all-bf16/fp16 matmuls, q-major AV):
  - QT/KT = (q @ Wq|k)^T in [dh2h, bs] layout (lhsT = W chunk, rhs = qT chunk)
  - V projected directly into vh [s(=k), dv2h] tiles (lhsT = qT chunk)
  - scores k-major: sT[k, q] = KT_h-slice.T @ QT_h-slice, psum [128, 1024]
  - exp on ScalarE (fused 1/8 scale, no max subtraction; scores ~N(0,1)),
    strip bf16 in SBUF
  - AV q-major: ctx[q, dv] += strip_slice.T @ vh_slice accumulated over the
    16 k-tiles; row-sums via an extra ones-column matmul per q-tile (out
    free size 1 -> ~free)
  - normalization fused into the ctx transpose: a regular matmul against
    diag(1/D) (built by GPSIMD from an fp16 identity) yields
    ctxT[dvh, q] = ctx[q, dvh]/D_q
  - out projection: po[q, e] = ctxT.T-slice @ Wo-slice, fp16 partial out

Scheduling: a global 128-slot pipeline (slot = one (pass, k-tile)); each slot
carries scores+exp+AV(lag 3) plus "filler" PE work (projections, V, out-proj)
budgeted so the tensor engine never idles (the cost model's p-state ramp
penalizes every PE idle gap). Warmup dummy matmuls cover the initial DMA wait.
"""

import numpy as np
import ml_dtypes

import concourse.bass as bass
import concourse.mybir as mybir
import concourse.tile as tile
from concourse import bacc
from concourse.bass_utils import run_bass_kernel_spmd
from concourse.masks import make_identity

BF16 = mybir.dt.bfloat16
FP16 = mybir.dt.float16
F32 = mybir.dt.float32
EXP = mybir.ActivationFunctionType.Exp

B, S, D = 2, 2048, 1024
H, DH = 16, 64
NCORES = 8
BS = B * S  # 4096
NCH = D // 128  # 8 contraction chunks for the projections
NKT = S // 128  # 16 k-tiles per batch
NQT = 8  # q-tiles (128) per q-half
AV_LAG = 3  # slots between exp(s) and AV(s)
N_DUMMY = 30  # warmup matmuls riding out the DMA wait + p-state ramp

# pass p = (b, qh, h); slot s = p*16 + kt
PASSES = [(b, qh, h) for b in range(B) for qh in range(2) for h in range(2)]
NSLOT = len(PASSES) * NKT  # 128

_CACHED_NC = None


def _build():
    nc = bacc.Bacc("TRN2", target_bir_lowering=False, debug=False)

    qt_d = nc.dram_tensor("qt", [128, NCH, BS], BF16, kind="ExternalInput")
    wq_d = nc.dram_tensor("wq", [128, NCH, 128], BF16, kind="ExternalInput")
    wk_d = nc.dram_tensor("wk", [128, NCH, 128], BF16, kind="ExternalInput")
    wv_d = nc.dram_tensor("wv", [128, NCH, 128], BF16, kind="ExternalInput")
    wo_d = nc.dram_tensor("wo", [128, D], FP16, kind="ExternalInput")
    out_d = nc.dram_tensor("out", [BS, D], FP16, kind="ExternalOutput")

    with tile.TileContext(nc) as tc:
        with (
            tc.tile_pool(name="cp", bufs=1) as cp,
            tc.tile_pool(name="sp", bufs=1) as sp,
            tc.tile_pool(name="ps", bufs=1, space="PSUM") as ps,
        ):
            # ---- persistent SBUF ----
            qt_sb = cp.tile([128, NCH, BS], BF16, tag="qt")
            wq_sb = cp.tile([128, NCH, 128], BF16, tag="wq")
            wk_sb = cp.tile([128, NCH, 128], BF16, tag="wk")
            wv_sb = cp.tile([128, NCH, 128], BF16, tag="wv")
            wo_sb = cp.tile([128, D], FP16, tag="wo")
            QTt = cp.tile([128, BS], BF16, tag="QT")  # [2h*dh, b*s]
            KTt = cp.tile([128, BS], BF16, tag="KT")
            vht = cp.tile([128, B, NKT, 128], BF16, tag="vh")  # [k, b, kt, dv2h]
            onesc = cp.tile([128, 1], BF16, tag="ones")
            identf = cp.tile([128, 128], FP16, tag="ident")
            dmy_w = cp.tile([128, 128], BF16, tag="dmy_w")

            # one shared PSUM bank for the small tiles (bank-granular
            # allocator): per-kt D partials [*,0:128] as [8qt,16kt], vv
            # [*,128:256], ptr ping-pong [*,256:384] / [*,384:512]
            smallp = ps.tile([128, 512], F32, tag="small", bufs=1)

            nc.vector.memset(onesc[:], 1.0)
            nc.vector.memset(dmy_w[:], 0.0)
            make_identity(nc, identf[:])

            # ---- warmup dummies (PE busy during DMA wait; ride the ramp) ----
            for i in range(N_DUMMY):
                pd = ps.tile(
                    [128, 1024], F32, tag="pscr", bufs=2, name=f"dmy{i}"
                )
                nc.tensor.matmul(
                    pd[:, 0:128], dmy_w[:], dmy_w[:], start=True, stop=True
                )

            # ---- input DMAs: critical wave first, issue spread across
            # the SP/ACT/DVE DGE queues (issue itself serializes per engine)
            nc.sync.dma_start(wq_sb[:], wq_d.ap())
            nc.scalar.dma_start(wk_sb[:], wk_d.ap())
            nc.sync.dma_start(qt_sb[:, 0:4, 0:512], qt_d[:, 0:4, 0:512])
            nc.scalar.dma_start(qt_sb[:, 4:8, 0:512], qt_d[:, 4:8, 0:512])
            nc.sync.dma_start(qt_sb[:, 0:4, 512:1024], qt_d[:, 0:4, 512:1024])
            nc.scalar.dma_start(qt_sb[:, 4:8, 512:1024], qt_d[:, 4:8, 512:1024])
            nc.sync.dma_start(wv_sb[:], wv_d.ap())
            nc.scalar.dma_start(qt_sb[:, 4:8, 1024:2048], qt_d[:, 4:8, 1024:2048])
            nc.sync.dma_start(qt_sb[:, 0:4, 1024:2048], qt_d[:, 0:4, 1024:2048])
            nc.scalar.dma_start(qt_sb[:, 4:8, 2048:3072], qt_d[:, 4:8, 2048:3072])
            nc.sync.dma_start(qt_sb[:, 0:4, 2048:3072], qt_d[:, 0:4, 2048:3072])
            nc.scalar.dma_start(qt_sb[:, 4:8, 3072:BS], qt_d[:, 4:8, 3072:BS])
            nc.sync.dma_start(qt_sb[:, 0:4, 3072:BS], qt_d[:, 0:4, 3072:BS])
            nc.scalar.dma_start(wo_sb[:], wo_d[:, :])

            # ---- helpers ----
            def proj_half(which, blk, half):
                """4 of the 8 contraction-chunk matmuls of one 512-col
                projection block. Each half owns its flex psum tile for its
                whole lifecycle (alloc->mms->evict) so the shared flex ring
                can never deadlock the PE FIFO. half 1 adds into the dst."""
                w_sb, dst = (wq_sb, QTt) if which == "Q" else (wk_sb, KTt)
                s0 = blk * 512
                pt = ps.tile(
                    [128, 512], F32, tag="flex", bufs=2, name=f"pt_{which}{blk}_{half}"
                )
                for c in range(half * 4, half * 4 + 4):
                    nc.tensor.matmul(
                        pt[:],
                        w_sb[:, c, :],
                        qt_sb[:, c, s0 : s0 + 512],
                        start=(c == half * 4),
                        stop=(c == half * 4 + 3),
                    )
                if half == 0:
                    nc.vector.tensor_copy(dst[:, s0 : s0 + 512], pt[:])
                else:
                    nc.vector.tensor_tensor(
                        dst[:, s0 : s0 + 512],
                        dst[:, s0 : s0 + 512],
                        pt[:],
                        mybir.AluOpType.add,
                    )

            def v_tile(bb, st):
                """Project one 128-row s-tile of batch bb into vh."""
                vvt = ps.tile([128, 512], F32, tag="flex", bufs=2, name=f"vv{bb}_{st}")
                vv = vvt[:, 0:128]
                s0 = bb * S + st * 128
                for c in range(NCH):
                    nc.tensor.matmul(
                        vv[:],
                        qt_sb[:, c, s0 : s0 + 128],
                        wv_sb[:, c, :],
                        start=(c == 0),
                        stop=(c == 7),
                    )
                nc.vector.tensor_copy(vht[:, bb, st, :], vv[:])

            cc_tiles = {}
            rcp_tiles = {}
            diag_tiles = {}
            ptr_idx = [0]

            def scores_exp(b, qh, h, kt, s):
                hp = h * 64
                k0 = b * S + kt * 128
                q0 = b * S + qh * 1024
                pscr = ps.tile([128, 1024], F32, tag="pscr", bufs=2, name=f"sc{s}")
                for j in range(2):
                    nc.tensor.matmul(
                        pscr[:, j * 512 : (j + 1) * 512],
                        KTt[hp : hp + 64, k0 : k0 + 128],
                        QTt[hp : hp + 64, q0 + j * 512 : q0 + (j + 1) * 512],
                        start=True,
                        stop=True,
                    )
                strip = sp.tile(
                    [128, 1024], BF16, tag="strip", bufs=32, name=f"st{s}"
                )
                nc.scalar.activation(strip[:], pscr[:], EXP, scale=0.125)
                return strip

            av_state = {}
            strips = {}

            def av_qt(p, qtt):
                """Full 16-kt accumulation for one q-tile of pass p (runs as
                a filler during pass p+1). Groups are contiguous: interleaved
                PSUM accumulation groups in one bank break (bank-level
                has_written clear on start)."""
                b, qh, h = PASSES[p]
                if qtt == 0:
                    av_state[p] = ps.tile(
                        [128, NQT, 64], F32, tag="avp", bufs=1, name=f"av{p}"
                    )
                avp = av_state[p]
                for kt in range(NKT):
                    nc.tensor.matmul(
                        avp[:, qtt, :],
                        strips[p * 16 + kt][:, qtt * 128 : (qtt + 1) * 128],
                        vht[:, b, kt, h * 64 : h * 64 + 64],
                        start=(kt == 0),
                        stop=(kt == NKT - 1),
                    )
                for kt in range(NKT):
                    nc.tensor.matmul(
                        smallp[:, qtt : qtt + 1],
                        strips[p * 16 + kt][:, qtt * 128 : (qtt + 1) * 128],
                        onesc[:],
                        start=(kt == 0),
                        stop=(kt == NKT - 1),
                    )
                if qtt == NQT - 1:
                    for kt in range(NKT):
                        strips.pop(p * 16 + kt)
                    drain(p)

            def drain(p):
                """recip of the row-sums + evacuate ctx into cc (SBUF fp16)."""
                b, qh, h = PASSES[p]
                avp = av_state.pop(p)
                if h == 0:
                    cc_tiles[(b, qh)] = cp.tile(
                        [128, NQT, 128], FP16, tag="cc", bufs=4, name=f"cc{b}{qh}"
                    )
                    rcp_tiles[(b, qh)] = cp.tile(
                        [128, 2, NQT], F32, tag="rcp", bufs=4, name=f"rcp{b}{qh}"
                    )
                cc = cc_tiles[(b, qh)]
                rcp = rcp_tiles[(b, qh)]
                with nc.allow_low_precision(reason="softmax denominator"):
                    nc.vector.reciprocal(rcp[:, h, :], smallp[:, 0:8])
                nc.vector.tensor_copy(cc[:, :, h * 64 : h * 64 + 64], avp[:])
                if h == 1:
                    # pre-build all diag(1/D) tiles on GPSIMD so the
                    # out-projection transposes never wait on Pool
                    dgs = []
                    for qtt in range(NQT):
                        for hh in range(2):
                            dg = sp.tile(
                                [128, 128], FP16, tag="diag", bufs=36,
                                name=f"dg{b}{qh}{qtt}{hh}",
                            )
                            nc.gpsimd.tensor_scalar_mul(
                                dg[:], identf[:], rcp[:, hh, qtt : qtt + 1]
                            )
                            dgs.append(dg)
                    diag_tiles[(b, qh)] = dgs

            def outproj_a(b, qh, qtt):
                """transpose-normalize matmuls (diag pre-built) + ctxT copy."""
                cc = cc_tiles[(b, qh)]
                pp = ptr_idx[0] % 2
                ptr_idx[0] += 1
                ptr = smallp[:, 256 + pp * 128 : 384 + pp * 128]
                for h in range(2):
                    dg = diag_tiles[(b, qh)][qtt * 2 + h]
                    nc.tensor.matmul(
                        ptr[h * 64 : (h + 1) * 64, :],
                        cc[:, qtt, h * 64 : (h + 1) * 64],
                        dg[:],
                        start=True,
                        stop=True,
                    )
                ctxT = sp.tile(
                    [128, 128], FP16, tag="ctxT", bufs=6, name=f"cx{b}{qh}{qtt}"
                )
                nc.vector.tensor_copy(ctxT[:], ptr[:])
                return ctxT

            def outproj_b(b, qh, qtt, ctxT, eh, tail=False):
                po = ps.tile(
                    [128, 512], F32, tag="flex", bufs=2, name=f"po{b}{qh}{qtt}{eh}"
                )
                nc.tensor.matmul(
                    po[:],
                    ctxT[:],
                    wo_sb[:, eh * 512 : (eh + 1) * 512],
                    start=True,
                    stop=True,
                )
                ob = sp.tile(
                    [128, 512], FP16, tag="ob", bufs=3, name=f"ob{b}{qh}{qtt}{eh}"
                )
                if tail and eh == 1:
                    nc.scalar.copy(ob[:], po[:])  # ACT is free in the tail
                else:
                    nc.vector.tensor_copy(ob[:], po[:])
                r0 = b * S + qh * 1024 + qtt * 128
                nc.sync.dma_start(
                    out_d[r0 : r0 + 128, eh * 512 : (eh + 1) * 512], ob[:]
                )

            # ---- filler schedule ----
            # Each filler is (ready_slot, deadline_slot, fn). Deadlines are
            # asserted; ready respects data deps (proj blocks are sequential
            # through the single pt psum tile by construction of the list).
            fillers = []

            def F(ready, deadline, fn, cost=450):
                fillers.append([ready, deadline, fn, cost])

            # KT b0 remaining blocks (blk0 in prologue): scores kt needs KT
            # block kt//4 at slot kt.
            for blk, dl in ((1, 4), (2, 8), (3, 12)):
                for hh in range(2):
                    F(0, dl - 1, lambda blk=blk, hh=hh: proj_half("K", blk, hh), 853)
            # V b0: vh[st] needed by AV kt=st at slot st+AV_LAG (st0,1 in prologue)
            for st in range(2, 16):
                F(0, 14, lambda st=st: v_tile(0, st), 427)
            # QT b0 blk2,3 (qh1 -> slot 32)
            for blk in (2, 3):
                for hh in range(2):
                    F(4, 31, lambda blk=blk, hh=hh: proj_half("Q", blk, hh), 853)
            # b1 projections (scores from slot 64; KT block (kt//4)+4 at slot
            # 64+kt; QT blk4,5 by 64, blk6,7 by 96)
            for blk, dl in ((4, 62), (5, 66), (6, 70), (7, 74)):
                for hh in range(2):
                    F(16, dl, lambda blk=blk, hh=hh: proj_half("K", blk, hh), 853)
            for blk, dl in ((4, 62), (5, 62), (6, 93), (7, 93)):
                for hh in range(2):
                    F(
                        18,
                        dl,
                        lambda blk=blk, hh=hh: proj_half("Q", blk, hh),
                        853,
                    )
            # V b1: needed from slot 64+st+AV_LAG
            for st in range(16):
                F(40, 78, lambda st=st: v_tile(1, st), 427)

            # out-projections become ready once both h-passes of (b, qh) have
            # drained: pass p=(b,qh,1) drains at slot p*16+15+AV_LAG.
            ctxT_holder = {}

            def op_a(b, qh, qtt):
                ctxT_holder[(b, qh, qtt)] = outproj_a(b, qh, qtt)

            def op_b(b, qh, qtt, eh):
                outproj_b(b, qh, qtt, ctxT_holder[(b, qh, qtt)], eh)

            # deferred AV: pass p's per-q-tile accumulations run during
            # pass p+1 (they need all 16 strips of pass p)
            for p in range(len(PASSES) - 1):
                for qtt in range(NQT):
                    F(
                        (p + 1) * 16 + qtt,
                        (p + 1) * 16 + 13,
                        lambda p=p, qtt=qtt: av_qt(p, qtt),
                        500,
                    )

            for gi, (b, qh) in enumerate([(0, 0), (0, 1), (1, 0)]):
                rdy = (PASSES.index((b, qh, 1)) + 1) * 16 + 15
                for qtt in range(NQT):
                    F(rdy + 2 * qtt, NSLOT - 1, lambda b=b, qh=qh, qtt=qtt: op_a(b, qh, qtt), 260)
                for qtt in range(NQT):
                    F(rdy + 2 * qtt + 4, NSLOT - 1, lambda b=b, qh=qh, qtt=qtt: op_b(b, qh, qtt, 0), 260)
                    F(rdy + 2 * qtt + 5, NSLOT - 1, lambda b=b, qh=qh, qtt=qtt: op_b(b, qh, qtt, 1), 260)

            fillers.sort(key=lambda f: (f[0], f[1]))

            # ---- prologue projections (needed before slot 0) ----
            for hh in range(2):
                proj_half("Q", 0, hh)
            for hh in range(2):
                proj_half("K", 0, hh)
            for hh in range(2):
                proj_half("Q", 1, hh)
            v_tile(0, 0)
            v_tile(0, 1)

            # ---- main pipeline ----
            done = [False] * len(fillers)

            def pop_fillers(s, budget):
                """Emit due/ready fillers for slot s up to ~budget ns of PE
                work (earliest-deadline-first among ready; deadline-urgent
                entries are always emitted)."""
                spent = 0
                ready = [
                    (fillers[j][1], j)
                    for j in range(len(fillers))
                    if not done[j] and fillers[j][0] <= s
                ]
                ready.sort()
                for _, idx in ready:
                    rdy, dl, fn, cost = fillers[idx]
                    urgent = dl <= s + 2
                    if not urgent and spent + cost > budget + 200:
                        break
                    fn()
                    done[idx] = True
                    spent += cost
                return spent

            for s in range(NSLOT):
                p, kt = divmod(s, NKT)
                b, qh, h = PASSES[p]
                strips[s] = scores_exp(b, qh, h, kt, s)
                pop_fillers(s, budget=650)

            # ---- epilogue: phase 1 (AV/D + recip + cc + diags), then
            # phase 2 (transpose-normalize + out-proj), so PE never waits on
            # the DVE/Pool round-trips
            for idx, (rdy, dl, fn, cost) in enumerate(fillers):
                if not done[idx]:
                    fn()
                    done[idx] = True
            p7 = len(PASSES) - 1
            b7, qh7, h7 = PASSES[p7]
            cc7 = cc_tiles[(b7, qh7)]
            rcp7 = rcp_tiles[(b7, qh7)]
            avp7 = ps.tile([128, NQT, 64], F32, tag="avp", bufs=1, name="av7")
            dgs7 = []
            for qtt in range(NQT):
                for kt in range(NKT):
                    nc.tensor.matmul(
                        avp7[:, qtt, :],
                        strips[p7 * 16 + kt][:, qtt * 128 : (qtt + 1) * 128],
                        vht[:, b7, kt, h7 * 64 : h7 * 64 + 64],
                        start=(kt == 0),
                        stop=(kt == NKT - 1),
                    )
                for kt in range(NKT):
                    nc.tensor.matmul(
                        smallp[:, qtt : qtt + 1],
                        strips[p7 * 16 + kt][:, qtt * 128 : (qtt + 1) * 128],
                        onesc[:],
                        start=(kt == 0),
                        stop=(kt == NKT - 1),
                    )
                with nc.allow_low_precision(reason="softmax denominator"):
                    nc.vector.reciprocal(
                        rcp7[:, h7, qtt : qtt + 1], smallp[:, qtt : qtt + 1]
                    )
                nc.vector.tensor_copy(
                    cc7[:, qtt, h7 * 64 : h7 * 64 + 64], avp7[:, qtt, :]
                )
                for hh in range(2):
                    dg = sp.tile(
                        [128, 128], FP16, tag="diag", bufs=36, name=f"edg{qtt}{hh}"
                    )
                    nc.gpsimd.tensor_scalar_mul(
                        dg[:], identf[:], rcp7[:, hh, qtt : qtt + 1]
                    )
                    dgs7.append(dg)
            for qtt in range(NQT):
                pp = ptr_idx[0] % 2
                ptr_idx[0] += 1
                ptr = smallp[:, 256 + pp * 128 : 384 + pp * 128]
                for h in range(2):
                    nc.tensor.matmul(
                        ptr[h * 64 : (h + 1) * 64, :],
                        cc7[:, qtt, h * 64 : (h + 1) * 64],
                        dgs7[qtt * 2 + h][:],
                        start=True,
                        stop=True,
                    )
                ctxT = sp.tile([128, 128], FP16, tag="ctxT", bufs=6, name=f"ecx{qtt}")
                nc.vector.tensor_copy(ctxT[:], ptr[:])
                po = ps.tile([128, 1024], F32, tag="pscr", bufs=2, name=f"epo{qtt}")
                for eh in range(2):
                    nc.tensor.matmul(
                        po[:, eh * 512 : (eh + 1) * 512],
                        ctxT[:],
                        wo_sb[:, eh * 512 : (eh + 1) * 512],
                        start=True,
                        stop=True,
                    )
                ob = sp.tile([128, 1024], FP16, tag="eob", bufs=3, name=f"eob{qtt}")
                nc.vector.tensor_copy(ob[:, 0:512], po[:, 0:512])
                nc.scalar.copy(ob[:, 512:1024], po[:, 512:1024])
                r0 = b7 * S + qh7 * 1024 + qtt * 128
                nc.sync.dma_start(out_d[r0 : r0 + 128, :], ob[:])
            for kt in range(NKT):
                strips.pop(p7 * 16 + kt)

    nc.compile()
    return nc


def _get_nc():
    global _CACHED_NC
    if _CACHED_NC is None:
        _CACHED_NC = _build()
    return _CACHED_NC


def _in_maps(q, Wq, Wk, Wv, Wo):
    q32 = np.asarray(q, np.float32).reshape(BS, NCH, 128)
    qt = np.ascontiguousarray(q32.transpose(2, 1, 0)).astype(ml_dtypes.bfloat16)

    def warr(W, sl):
        w = np.asarray(W, np.float32)[:, sl].reshape(NCH, 128, 128).transpose(1, 0, 2)
        return np.ascontiguousarray(w).astype(ml_dtypes.bfloat16)

    Wo32 = np.asarray(Wo, np.float32)
    maps = []
    for c in range(NCORES):
        sl = slice(c * 128, (c + 1) * 128)
        maps.append(
            {
                "qt": qt,
                "wq": warr(Wq, sl),
                "wk": warr(Wk, sl),
                "wv": warr(Wv, sl),
                "wo": np.ascontiguousarray(Wo32[sl, :]).astype(np.float16),
            }
        )
    return maps


def run(q, Wq, Wk, Wv, Wo, bo, trace=False):
    nc = _get_nc()
    res = run_bass_kernel_spmd(
        nc, _in_maps(q, Wq, Wk, Wv, Wo), list(range(NCORES)), trace=trace
    )
    acc = np.zeros((BS, D), np.float32)
    for r in res.results:
        acc += r["out"].astype(np.float32)
    out = (acc + np.asarray(bo, np.float32)).astype(np.float32)
    return out.reshape(B, S, D), res


def kernel(q, Wq, Wk, Wv, Wo, bo):
    out, _ = run(q, Wq, Wk, Wv, Wo, bo)
    return out
